# revision 10
# baseline (speedup 1.0000x reference)
"""Trainium2 Bass kernel for nn_BasicBlock (binary activation + binarized
weight-standardized 3x3 conv + residual + PReLU).

Contract: kernel(**inputs) takes FULL unsharded numpy inputs (keys as in
setup_inputs) and returns the FULL [32, 512, 28, 28] float32 output.
Internally shards the batch dim across 8 NeuronCores (4 images each); the
small conv weight + per-channel vectors are replicated.

Key math facts exploited:
- forward activations are sign(x*beta+b0) in {-1,0,1} and forward weights
  are sf[o]*gain[o]*sign(w_std) with sign in {-1,0,1}, so the conv
  contraction is exact in fp8 (products are +-1, fp32 PSUM accumulation);
  the per-channel scalar alpha*sf*gain folds into the epilogue.
- fp8e4 DoubleRow packs two contraction rows per PE cell (2 cin chunks per
  matmul), halving the matmul count.
- epilogue uses PReLU(v) = max(v, a*v) (valid for 0<=a<=1):
  out = max(z + b1 + b2, a*z + a*b1 + b2) with z = conv*alphabar + x,
  computed as z on DVE, u/o on GpSimd.

Schedule (from trace analysis of the 157us baseline):
- w chunk0 DMA pieces + x img0 land first (one logical queue each; a
  single dma_start is split across all 16 SDMA engines, so few big pieces
  beat many small ones; trigger cost ~0.6us each on the issuing engine).
- per-piece bn_stats pipeline with the DMA; wsign blocks 0,1 unlock the
  first transposes; casts (q0 on DVE, q1 on ACT) feed lhsT per (tap,pair)
  so the first conv matmul fires ~8us into the window instead of ~27us.
- conv loop is n-outer, h2-inner: each (q,t) weight is reused for 2
  matmuls so LDWEIGHTS always hides under the matmul stream.
- xsign: ACT for most; (n0,c2),(n0,c3),(n1,c2),(n1,c3) on GpSimd via a
  (x+b0)*1e30 clamp so ACT's wsign/abs stream isn't on the act critical
  path.
- weight prep for chunk m+1 is hooked into chunk m's conv at n==1/n==2.
"""

import numpy as np

import concourse.bass as bass
import concourse.mybir as mybir
import concourse.tile as tile
from concourse import bacc
from concourse.masks import make_identity

# problem constants (hardcoded per harness contract)
N_CORES = 8
N_PER = 4          # images per core (32 / 8)
C = 512            # Cin == Cout
H = W = 28
HP = WP = 30       # zero-padded spatial
TAPS = 9
KFAN = C * TAPS    # 4608 = fan-in per output channel
ALPHA = 0.2
BETA = 1.0
EPS = 1e-5
WS_SCALE = 1.0 / float(np.sqrt(KFAN))  # fan_in**-0.5
NCH = C // 128     # 4 channel chunks of 128
NPAIR = NCH // 2   # 2 DoubleRow pairs of chunks
ROWS_PER_TILE = 14 # output rows per matmul tile
NSPAT = H // ROWS_PER_TILE  # 2 spatial tiles per image
ACT_IMG = 912  # padded 30x30 image (900) + 12 slack: %16==0 for DoubleRow,
               # and covers the last tile's 420-run overhang
BIG = 1e30     # sign-via-clamp scale for the GpSimd xsign path

FP32 = mybir.dt.float32
BF16 = mybir.dt.bfloat16
FP8 = mybir.dt.float8e4

# weight DMA piece layout: 1024-column pieces (stats slices of 512 wait on
# piece j//2) -> 5 triggers per chunk instead of 9
W_PIECES = [(0, 1024), (1024, 1024), (2048, 1024), (3072, 1024), (4096, 512)]


def build_program():
    nc = bacc.Bacc(
        "TRN2",
        target_bir_lowering=False,
        debug=False,
        num_devices=1,
        num_swdge_queues=4,
    )
    x_h = nc.declare_dram_parameter("x", [N_PER, C, H, W], FP32, isOutput=False)
    w_h = nc.declare_dram_parameter("conv_weight", [C, C, 3, 3], FP32, isOutput=False)
    gain_h = nc.declare_dram_parameter("gain", [C], FP32, isOutput=False)
    b0_h = nc.declare_dram_parameter("move0_bias", [C], FP32, isOutput=False)
    b1_h = nc.declare_dram_parameter("move1_bias", [C], FP32, isOutput=False)
    pa_h = nc.declare_dram_parameter("prelu_a", [C], FP32, isOutput=False)
    b2_h = nc.declare_dram_parameter("move2_bias", [C], FP32, isOutput=False)
    out_h = nc.declare_dram_parameter("out", [N_PER, C, H, W], FP32, isOutput=True)

    x_ap = x_h[:, :, :, :]
    w_ap = w_h[:, :, :, :]
    out_ap = out_h[:, :, :, :]

    with tile.TileContext(nc) as tc:
        with (
            tc.tile_pool(name="persist", bufs=1) as persist,
            tc.tile_pool(name="scratch", bufs=2) as scratch,
            tc.tile_pool(name="stats", bufs=4) as stats,
            tc.tile_pool(name="epi", bufs=4) as epi,
            tc.tile_pool(name="psum_mm", bufs=6, space="PSUM") as psum_mm,
            tc.tile_pool(name="psum_tr", bufs=2, space="PSUM") as psum_tr,
        ):
            # ---- w chunk 0 DMA first: the whole kernel gates on it ------
            w_flat = w_ap.rearrange("o i a b -> o (i a b)")
            w_tiles = []

            def w_dma(m):
                wt = scratch.tile([128, KFAN], FP32, tag="wtile", name=f"wt{m}")
                for (c0, cw) in W_PIECES:
                    nc.sync.dma_start(
                        out=wt[:, c0 : c0 + cw],
                        in_=w_flat[m * 128 : (m + 1) * 128, c0 : c0 + cw],
                    )
                w_tiles.append(wt)

            w_dma(0)

            # ---- gpsimd: identity, act borders (img0), x img0 ------------
            ident = persist.tile([128, 128], BF16, tag="ident")
            make_identity(nc, ident)

            act_img = []
            for q in range(NPAIR):
                row = []
                for n in range(N_PER):
                    ap_t = persist.tile(
                        [128, 2, ACT_IMG], FP8, tag=f"act{q}_{n}", name=f"act{q}_{n}"
                    )
                    row.append(ap_t)
                act_img.append(row)

            def act_memset(q, n):
                ap_t = act_img[q][n]
                nc.gpsimd.memset(ap_t[:, :, 0:WP], 0.0)
                nc.gpsimd.memset(ap_t[:, :, 29 * WP : ACT_IMG], 0.0)
                mid = ap_t[:, :, WP : 29 * WP].rearrange("p h (r c) -> p h r c", c=WP)
                nc.gpsimd.memset(mid[:, :, :, 0:1], 0.0)
                nc.gpsimd.memset(mid[:, :, :, 29:30], 0.0)

            for q in range(NPAIR):
                act_memset(q, 0)

            xs_all = persist.tile([128, NCH, N_PER, H, W], FP32, tag="xs", name="xs")
            xr = x_ap.rearrange("n (cc p) h w -> p cc n (h w)", p=128)

            def x_dma(n, chalf):  # chalf: 0 -> chunks 0,1 ; 1 -> chunks 2,3
                c0 = 2 * chalf
                nc.gpsimd.dma_start(
                    out=xs_all[:, c0 : c0 + 2, n].rearrange("p c h w -> p c (h w)"),
                    in_=xr[:, c0 : c0 + 2, n, :],
                )

            x_dma(0, 0)
            x_dma(0, 1)

            # ---- sync: per-channel vectors (one DMA each), then w 1-3 ----
            def load_vec(dram_h, name):
                t = persist.tile([128, NCH], FP32, tag=name, name=name)
                nc.sync.dma_start(
                    out=t, in_=dram_h[:].rearrange("(c p) -> p c", p=128)
                )
                return [t[:, c : c + 1] for c in range(NCH)]

            gain_c = load_vec(gain_h, "gain")
            b0_c = load_vec(b0_h, "b0")
            b1_c = load_vec(b1_h, "b1")
            pa_c = load_vec(pa_h, "pa")
            b2_c = load_vec(b2_h, "b2")

            for m in range(1, NCH):
                w_dma(m)

            # gpsimd: x images 1-3 (split 2/3 for finer completion)
            x_dma(1, 0)
            x_dma(1, 1)

            # ---- weight stats chunk 0 (DVE; slices pipeline with DMA) ----
            lhsT = persist.tile(
                [128, TAPS, NPAIR, 2, C], FP8, tag="lhsT", name="lhsT"
            )
            alphabar = {}
            wsigns = {}
            mvs = {}

            def weight_stats(m):
                wt = w_tiles[m]
                st = stats.tile([128, TAPS, 6], FP32, tag="bnst", name="bnst")
                wt3 = wt.rearrange("p (a b) -> p a b", b=512)
                for sg in range(TAPS):
                    nc.vector.bn_stats(out=st[:, sg, :], in_=wt3[:, sg, :])
                mv = stats.tile([128, 2], FP32, tag="bnagg", name="bnagg")
                nc.vector.bn_aggr(out=mv, in_=st)
                negmean = stats.tile([128, 1], FP32, tag="negmean", name="negmean")
                nc.vector.tensor_scalar_mul(out=negmean, in0=mv[:, 0:1], scalar1=-1.0)
                mvs[m] = (mv, negmean)

            def wsign_block(m, b):
                # sign(w - mean) -> bf16 for cin block b
                if b == 0:
                    ws = scratch.tile([128, KFAN], BF16, tag="wsign", name=f"ws{m}")
                    wsigns[m] = ws
                ws = wsigns[m]
                nc.scalar.activation(
                    out=ws[:, b * 1152 : (b + 1) * 1152],
                    in_=w_tiles[m][:, b * 1152 : (b + 1) * 1152],
                    func=mybir.ActivationFunctionType.Sign,
                    bias=mvs[m][1],
                )

            def stdeps_sqrt(m):
                sd = stats.tile([128, 1], FP32, tag="stdeps", name=f"sd{m}")
                nc.scalar.activation(
                    out=sd, in_=mvs[m][0][:, 1:2],
                    func=mybir.ActivationFunctionType.Sqrt,
                )
                mvs[m] = (mvs[m][0], mvs[m][1], sd)

            def wabs(m, blocks, sumabs_t):
                # |w - mean| accumulated per block (overwrites w tile)
                wt = w_tiles[m]
                for i, b in enumerate(blocks):
                    nc.scalar.activation(
                        out=wt[:, b * 1152 : (b + 1) * 1152],
                        in_=wt[:, b * 1152 : (b + 1) * 1152],
                        func=mybir.ActivationFunctionType.Abs,
                        bias=mvs[m][1],
                        accum_out=sumabs_t[:, b : b + 1],
                    )

            def alphabar_tail(m, sumabs_t, nblk):
                sd = mvs[m][2]
                sde = stats.tile([128, 1], FP32, tag="sde", name=f"sde{m}")
                nc.vector.tensor_scalar_add(out=sde, in0=sd, scalar1=EPS)
                inv = stats.tile([128, 1], FP32, tag="inv", name=f"inv{m}")
                nc.vector.reciprocal(out=inv, in_=sde)
                s1 = stats.tile([128, 1], FP32, tag="s1", name=f"s1{m}")
                if nblk > 1:
                    nc.vector.tensor_reduce(
                        out=s1, in_=sumabs_t[:, 0:nblk], axis=mybir.AxisListType.X,
                        op=mybir.AluOpType.add,
                    )
                else:
                    nc.vector.tensor_copy(out=s1, in_=sumabs_t[:, 0:1])
                ab = persist.tile(
                    [128, 1], FP32, tag=f"alphabar{m}", name=f"alphabar{m}"
                )
                nc.vector.tensor_tensor(
                    out=ab, in0=s1, in1=inv, op=mybir.AluOpType.mult
                )
                nc.vector.tensor_tensor(
                    out=ab, in0=ab, in1=gain_c[m], op=mybir.AluOpType.mult
                )
                nc.vector.tensor_scalar_mul(
                    out=ab, in0=ab, scalar1=ALPHA * WS_SCALE / KFAN
                )
                alphabar[m] = ab

            def transpose_cast(m, t, q, cast_engine):
                # 2 cin blocks (pair q) of tap t -> lhsT[:, t, q, :, m cols]
                ws3 = wsigns[m].rearrange("p (i t) -> p i t", t=TAPS)
                ps = psum_tr.tile([128, 2 * 128], BF16, tag="ptr", name="ptr")
                for h in range(2):
                    b = 2 * q + h
                    nc.tensor.transpose(
                        ps[:, h * 128 : (h + 1) * 128],
                        ws3[:, b * 128 : (b + 1) * 128, t],
                        ident,
                    )
                dst = lhsT[:, t, q, :, m * 128 : (m + 1) * 128]
                if cast_engine == "vector":
                    nc.vector.tensor_copy(out=dst, in_=ps)
                else:
                    nc.scalar.activation(
                        out=dst, in_=ps,
                        func=mybir.ActivationFunctionType.Copy,
                    )

            # ---- xsign: ACT path and GpSimd clamp path -------------------
            def xsign_act(n, c):
                dst = act_img[c // 2][n][:, c % 2, : HP * WP].rearrange(
                    "p (h w) -> p h w", w=WP
                )[:, 1 : 1 + H, 1 : 1 + W]
                nc.scalar.activation(
                    out=dst,
                    in_=xs_all[:, c, n],
                    func=mybir.ActivationFunctionType.Sign,
                    bias=b0_c[c],
                    scale=BETA,
                )

            def xsign_gps(n, c):
                dst = act_img[c // 2][n][:, c % 2, : HP * WP].rearrange(
                    "p (h w) -> p h w", w=WP
                )[:, 1 : 1 + H, 1 : 1 + W]
                tmp = epi.tile([128, H, W], FP32, tag="xtmp", name="xtmp")
                nc.gpsimd.tensor_scalar(
                    out=tmp, in0=xs_all[:, c, n], scalar1=b0_c[c], scalar2=BIG,
                    op0=mybir.AluOpType.add, op1=mybir.AluOpType.mult,
                )
                nc.gpsimd.tensor_scalar(
                    out=dst, in0=tmp, scalar1=1.0, scalar2=-1.0,
                    op0=mybir.AluOpType.min, op1=mybir.AluOpType.max,
                )

            # ---- HAM warmup: junk matmuls so the PE clock-gate opens to
            # 2.4 GHz before the real stream starts (transpose-mode does
            # not warm HAM, real MMs do; ~36 x 107ns fills the window
            # while the w chunk-0 DMA is in flight) ------------------------
            jp = psum_mm.tile(
                [128, ROWS_PER_TILE * W], FP32, tag="acc", name="junk"
            )
            for _ in range(36):
                nc.tensor.matmul(jp[:, :128], ident, ident, start=True, stop=True)

            # ---- prologue chain for chunk 0 ------------------------------
            # ACT order: img0 signs (x lands first), then weight signs,
            # then chunk0 q1 casts, then later-image signs, then abs.
            xsign_act(0, 0)
            xsign_act(0, 1)
            weight_stats(0)
            wsign_block(0, 0)
            wsign_block(0, 1)
            stdeps_sqrt(0)
            wsign_block(0, 2)
            wsign_block(0, 3)

            # transposes + casts: q0 via DVE, q1 via ACT
            for t in range(TAPS):
                transpose_cast(0, t, 0, "vector")
            for t in range(TAPS):
                transpose_cast(0, t, 1, "scalar")

            # gpsimd covers the c2/c3 act chunks for every image
            xsign_gps(0, 2)
            xsign_gps(0, 3)

            # x imgs 2-3 (gpsimd triggers, after the n0 clamp ops)
            x_dma(2, 0)
            x_dma(2, 1)
            x_dma(3, 0)
            x_dma(3, 1)

            # remaining act borders + xsign for n=1..3 (c01 ACT, c23 gps)
            for q in range(NPAIR):
                act_memset(q, 1)
            xsign_act(1, 0)
            xsign_act(1, 1)
            xsign_gps(1, 2)
            xsign_gps(1, 3)
            for q in range(NPAIR):
                act_memset(q, 2)
                act_memset(q, 3)
            xsign_act(2, 0)
            xsign_act(2, 1)
            xsign_gps(2, 2)
            xsign_gps(2, 3)

            # chunk0 |w-mean| on ACT after the n<=2 signs; alphabar gates
            # only the epilogue z ops, not the MM stream
            sumabs0 = stats.tile([128, NCH], FP32, tag="sumabs", name="sumabs0")
            wabs(0, [0, 1, 2, 3], sumabs0)
            alphabar_tail(0, sumabs0, NCH)

            xsign_act(3, 0)
            xsign_act(3, 1)
            xsign_gps(3, 2)
            xsign_gps(3, 3)

            # ---- conv ----------------------------------------------------
            def prep_a(m):
                # ACT part; weight_stats(m) is emitted earlier (DVE FIFO)
                for b in range(NCH):
                    wsign_block(m, b)
                stdeps_sqrt(m)
                sa = stats.tile([128, NCH], FP32, tag="sumabs", name=f"sa{m}")
                wabs(m, [0, 1, 2, 3], sa)
                alphabar_tail(m, sa, NCH)

            def prep_b(m):
                for t in range(TAPS):
                    transpose_cast(m, t, 0, "vector")
                    transpose_cast(m, t, 1, "vector")

            def conv_group(m, n):
                accs = []
                for h2 in range(NSPAT):
                    accs.append(
                        psum_mm.tile(
                            [128, ROWS_PER_TILE * W], FP32, tag="acc", name="acc"
                        )
                    )
                i = 0
                for q in range(NPAIR):
                    av = act_img[q][n][:, :, : HP * WP].rearrange(
                        "p h (r c) -> p h r c", c=WP
                    )
                    for t in range(TAPS):
                        dy, dx = t // 3, t % 3
                        for h2 in range(NSPAT):
                            y0 = h2 * ROWS_PER_TILE
                            rhs = av[
                                :, :, y0 + dy : y0 + dy + ROWS_PER_TILE,
                                dx : dx + W,
                            ]
                            nc.tensor.matmul(
                                accs[h2],
                                lhsT[:, t, q, :, m * 128 : (m + 1) * 128],
                                rhs,
                                start=(i == 0),
                                stop=(i == NPAIR * TAPS - 1),
                                perf_mode=mybir.MatmulPerfMode.DoubleRow,
                            )
                        i += 1
                for h2 in range(NSPAT):
                    y0 = h2 * ROWS_PER_TILE
                    accv = accs[h2].rearrange("p (h w) -> p h w", w=W)
                    res = xs_all[:, m, n, y0 : y0 + ROWS_PER_TILE, :]
                    # z = conv*alphabar + residual  (DVE, drains the bank)
                    z = epi.tile([128, ROWS_PER_TILE, W], FP32, tag="z", name="z")
                    nc.vector.scalar_tensor_tensor(
                        out=z, in0=accv, scalar=alphabar[m], in1=res,
                        op0=mybir.AluOpType.mult, op1=mybir.AluOpType.add,
                    )
                    # e = prelu(z + b1)  (ACT, per-channel alpha)
                    e = epi.tile([128, ROWS_PER_TILE, W], FP32, tag="e", name="e")
                    nc.scalar.activation(
                        out=e, in_=z,
                        func=mybir.ActivationFunctionType.Prelu,
                        bias=b1_c[m], alpha=pa_c[m],
                    )
                    # o = e + b2  (GpSimd)
                    o = epi.tile([128, ROWS_PER_TILE, W], FP32, tag="oo", name="oo")
                    nc.gpsimd.tensor_scalar(
                        out=o, in0=e, scalar1=b2_c[m], scalar2=None,
                        op0=mybir.AluOpType.add,
                    )
                    nsplit = 2 if (m == NCH - 1 and n == N_PER - 1
                                   and h2 == NSPAT - 1) else 1
                    rr = ROWS_PER_TILE // nsplit
                    for s in range(nsplit):
                        nc.sync.dma_start(
                            out=out_ap[
                                n, m * 128 : (m + 1) * 128,
                                y0 + s * rr : y0 + (s + 1) * rr, :,
                            ],
                            in_=o[:, s * rr : (s + 1) * rr],
                        )

            for m in range(NCH):
                if m + 1 < NCH:
                    weight_stats(m + 1)
                for n in range(N_PER):
                    conv_group(m, n)
                    if m + 1 < NCH:
                        if n == 0:
                            prep_a(m + 1)
                        elif n == 1:
                            prep_b(m + 1)

    nc.finalize()
    return nc


_NC_CACHE = None


def _get_program():
    global _NC_CACHE
    if _NC_CACHE is None:
        _NC_CACHE = build_program()
    return _NC_CACHE


def kernel(**inputs):
    from concourse.bass_utils import run_bass_kernel_spmd

    x = np.ascontiguousarray(np.asarray(inputs["x"], dtype=np.float32))
    shared = {
        name: np.ascontiguousarray(np.asarray(inputs[name], dtype=np.float32))
        for name in (
            "conv_weight", "gain", "move0_bias", "move1_bias", "prelu_a",
            "move2_bias",
        )
    }
    nc = _get_program()
    in_maps = [
        {"x": x[i * N_PER : (i + 1) * N_PER], **shared} for i in range(N_CORES)
    ]
    res = run_bass_kernel_spmd(nc, in_maps, core_ids=list(range(N_CORES)))
    return np.concatenate([r["out"] for r in res.results], axis=0)


# revision 14
# speedup vs baseline: 1.7307x; 1.7307x over previous
"""Trainium2 Bass kernel for nn_BasicBlock (binary activation + binarized
weight-standardized 3x3 conv + residual + PReLU).

Contract: kernel(**inputs) takes FULL unsharded numpy inputs (keys as in
setup_inputs) and returns the FULL [32, 512, 28, 28] float32 output.
Internally shards the batch dim across 8 NeuronCores (4 images each); the
small conv weight + per-channel vectors are replicated.

Key math facts exploited:
- forward activations are sign(x*beta+b0) in {-1,0,1} and forward weights
  are sf[o]*gain[o]*sign(w_std) with sign in {-1,0,1}, so the conv
  contraction is exact in fp8 (products are +-1, fp32 PSUM accumulation);
  the per-channel scalar alpha*sf*gain folds into the epilogue.
- fp8e4 DoubleRow packs two contraction rows per PE cell (2 cin chunks per
  matmul), halving the matmul count.
- epilogue uses PReLU(v) = max(v, a*v) (valid for 0<=a<=1):
  out = max(z + b1 + b2, a*z + a*b1 + b2) with z = conv*alphabar + x,
  computed as z on DVE, u/o on GpSimd.

Schedule (from trace analysis of the 157us baseline):
- w chunk0 DMA pieces + x img0 land first (one logical queue each; a
  single dma_start is split across all 16 SDMA engines, so few big pieces
  beat many small ones; trigger cost ~0.6us each on the issuing engine).
- per-piece bn_stats pipeline with the DMA; wsign blocks 0,1 unlock the
  first transposes; casts (q0 on DVE, q1 on ACT) feed lhsT per (tap,pair)
  so the first conv matmul fires ~8us into the window instead of ~27us.
- conv loop is n-outer, h2-inner: each (q,t) weight is reused for 2
  matmuls so LDWEIGHTS always hides under the matmul stream.
- xsign: ACT for most; (n0,c2),(n0,c3),(n1,c2),(n1,c3) on GpSimd via a
  (x+b0)*1e30 clamp so ACT's wsign/abs stream isn't on the act critical
  path.
- weight prep for chunk m+1 is hooked into chunk m's conv at n==1/n==2.
"""

import numpy as np

import concourse.bass as bass
import concourse.mybir as mybir
import concourse.tile as tile
from concourse import bacc
from concourse.masks import make_identity

# problem constants (hardcoded per harness contract)
N_CORES = 8
N_PER = 4          # images per core (32 / 8)
C = 512            # Cin == Cout
H = W = 28
HP = WP = 30       # zero-padded spatial
TAPS = 9
KFAN = C * TAPS    # 4608 = fan-in per output channel
ALPHA = 0.2
BETA = 1.0
EPS = 1e-5
WS_SCALE = 1.0 / float(np.sqrt(KFAN))  # fan_in**-0.5
NCH = C // 128     # 4 channel chunks of 128
NPAIR = NCH // 2   # 2 DoubleRow pairs of chunks
ROWS_PER_TILE = 14 # output rows per matmul tile
NSPAT = H // ROWS_PER_TILE  # 2 spatial tiles per image
ACT_IMG = 912  # padded 30x30 image (900) + 12 slack: %16==0 for DoubleRow,
               # and covers the last tile's 420-run overhang
BIG = 1e30     # sign-via-clamp scale for the GpSimd xsign path

FP32 = mybir.dt.float32
BF16 = mybir.dt.bfloat16
FP8 = mybir.dt.float8e4

# weight DMA piece layout: 1024-column pieces (stats slices of 512 wait on
# piece j//2) -> 5 triggers per chunk instead of 9
W_PIECES = [(0, 1024), (1024, 1024), (2048, 1024), (3072, 1024), (4096, 512)]


def build_program():
    nc = bacc.Bacc(
        "TRN2",
        target_bir_lowering=False,
        debug=False,
        num_devices=1,
        num_swdge_queues=4,
    )
    x_h = nc.declare_dram_parameter("x", [N_PER, C, H, W], FP32, isOutput=False)
    w_h = nc.declare_dram_parameter("conv_weight", [C, C, 3, 3], FP32, isOutput=False)
    gain_h = nc.declare_dram_parameter("gain", [C], FP32, isOutput=False)
    b0_h = nc.declare_dram_parameter("move0_bias", [C], FP32, isOutput=False)
    b1_h = nc.declare_dram_parameter("move1_bias", [C], FP32, isOutput=False)
    pa_h = nc.declare_dram_parameter("prelu_a", [C], FP32, isOutput=False)
    b2_h = nc.declare_dram_parameter("move2_bias", [C], FP32, isOutput=False)
    out_h = nc.declare_dram_parameter("out", [N_PER, C, H, W], FP32, isOutput=True)

    x_ap = x_h[:, :, :, :]
    w_ap = w_h[:, :, :, :]
    out_ap = out_h[:, :, :, :]

    with tile.TileContext(nc) as tc:
        with (
            tc.tile_pool(name="persist", bufs=1) as persist,
            tc.tile_pool(name="scratch", bufs=2) as scratch,
            tc.tile_pool(name="stats", bufs=4) as stats,
            tc.tile_pool(name="epi", bufs=4) as epi,
            tc.tile_pool(name="psum_mm", bufs=6, space="PSUM") as psum_mm,
            tc.tile_pool(name="psum_tr", bufs=2, space="PSUM") as psum_tr,
        ):
            # ---- w chunk 0 DMA first: the whole kernel gates on it ------
            w_flat = w_ap.rearrange("o i a b -> o (i a b)")
            w_tiles = []

            def w_dma(m):
                wt = scratch.tile([128, KFAN], FP32, tag="wtile", name=f"wt{m}")
                for (c0, cw) in W_PIECES:
                    nc.sync.dma_start(
                        out=wt[:, c0 : c0 + cw],
                        in_=w_flat[m * 128 : (m + 1) * 128, c0 : c0 + cw],
                    )
                w_tiles.append(wt)

            # tiny per-channel vectors go in front of w chunk0 on sync:
            # ~10KB total, and b0/gain gate the first xsign/alphabar
            def load_vec(dram_h, name):
                t = persist.tile([128, NCH], FP32, tag=name, name=name)
                nc.sync.dma_start(
                    out=t, in_=dram_h[:].rearrange("(c p) -> p c", p=128)
                )
                return [t[:, c : c + 1] for c in range(NCH)]

            gain_c = load_vec(gain_h, "gain")
            b0_c = load_vec(b0_h, "b0")
            b1_c = load_vec(b1_h, "b1")
            pa_c = load_vec(pa_h, "pa")
            b2_c = load_vec(b2_h, "b2")

            w_dma(0)

            # ---- gpsimd: identity, act borders (img0), x img0 ------------
            ident = persist.tile([128, 128], BF16, tag="ident")
            make_identity(nc, ident)

            act_img = []
            for q in range(NPAIR):
                row = []
                for n in range(N_PER):
                    ap_t = persist.tile(
                        [128, 2, ACT_IMG], FP8, tag=f"act{q}_{n}", name=f"act{q}_{n}"
                    )
                    row.append(ap_t)
                act_img.append(row)

            def act_memset(q, n):
                ap_t = act_img[q][n]
                nc.gpsimd.memset(ap_t[:, :, 0:WP], 0.0)
                nc.gpsimd.memset(ap_t[:, :, 29 * WP : ACT_IMG], 0.0)
                mid = ap_t[:, :, WP : 29 * WP].rearrange("p h (r c) -> p h r c", c=WP)
                nc.gpsimd.memset(mid[:, :, :, 0:1], 0.0)
                nc.gpsimd.memset(mid[:, :, :, 29:30], 0.0)

            for q in range(NPAIR):
                act_memset(q, 0)

            xs_all = persist.tile([128, NCH, N_PER, H, W], FP32, tag="xs", name="xs")
            xr = x_ap.rearrange("n (cc p) h w -> p cc n (h w)", p=128)

            def x_dma(n, chalf):  # chalf: 0 -> chunks 0,1 ; 1 -> chunks 2,3
                c0 = 2 * chalf
                nc.gpsimd.dma_start(
                    out=xs_all[:, c0 : c0 + 2, n].rearrange("p c h w -> p c (h w)"),
                    in_=xr[:, c0 : c0 + 2, n, :],
                )

            x_dma(0, 0)
            x_dma(0, 1)

            # ---- sync: w chunks 1-3 --------------------------------------
            for m in range(1, NCH):
                w_dma(m)

            # gpsimd: x images 1-3 (split 2/3 for finer completion)
            x_dma(1, 0)
            x_dma(1, 1)

            # ---- weight stats chunk 0 (DVE; slices pipeline with DMA) ----
            lhsT = persist.tile(
                [128, TAPS, NPAIR, 2, C], FP8, tag="lhsT", name="lhsT"
            )
            alphabar = {}
            wsigns = {}
            mvs = {}

            def weight_stats(m):
                wt = w_tiles[m]
                st = stats.tile([128, TAPS, 6], FP32, tag="bnst", name="bnst")
                wt3 = wt.rearrange("p (a b) -> p a b", b=512)
                for sg in range(TAPS):
                    nc.vector.bn_stats(out=st[:, sg, :], in_=wt3[:, sg, :])
                mv = stats.tile([128, 2], FP32, tag="bnagg", name="bnagg")
                nc.vector.bn_aggr(out=mv, in_=st)
                negmean = stats.tile([128, 1], FP32, tag="negmean", name="negmean")
                nc.vector.tensor_scalar_mul(out=negmean, in0=mv[:, 0:1], scalar1=-1.0)
                mvs[m] = (mv, negmean)

            def wsign_block(m, b):
                # sign(w - mean) -> bf16 for cin block b
                if b == 0:
                    ws = scratch.tile([128, KFAN], BF16, tag="wsign", name=f"ws{m}")
                    wsigns[m] = ws
                ws = wsigns[m]
                nc.scalar.activation(
                    out=ws[:, b * 1152 : (b + 1) * 1152],
                    in_=w_tiles[m][:, b * 1152 : (b + 1) * 1152],
                    func=mybir.ActivationFunctionType.Sign,
                    bias=mvs[m][1],
                )

            def stdeps_sqrt(m):
                sd = stats.tile([128, 1], FP32, tag="stdeps", name=f"sd{m}")
                nc.scalar.activation(
                    out=sd, in_=mvs[m][0][:, 1:2],
                    func=mybir.ActivationFunctionType.Sqrt,
                )
                mvs[m] = (mvs[m][0], mvs[m][1], sd)

            def wabs(m, blocks, sumabs_t):
                # |w - mean| accumulated per block (overwrites w tile)
                wt = w_tiles[m]
                for i, b in enumerate(blocks):
                    nc.scalar.activation(
                        out=wt[:, b * 1152 : (b + 1) * 1152],
                        in_=wt[:, b * 1152 : (b + 1) * 1152],
                        func=mybir.ActivationFunctionType.Abs,
                        bias=mvs[m][1],
                        accum_out=sumabs_t[:, b : b + 1],
                    )

            def alphabar_tail(m, sumabs_t, nblk):
                sd = mvs[m][2]
                sde = stats.tile([128, 1], FP32, tag="sde", name=f"sde{m}")
                nc.vector.tensor_scalar_add(out=sde, in0=sd, scalar1=EPS)
                inv = stats.tile([128, 1], FP32, tag="inv", name=f"inv{m}")
                nc.vector.reciprocal(out=inv, in_=sde)
                s1 = stats.tile([128, 1], FP32, tag="s1", name=f"s1{m}")
                if nblk > 1:
                    nc.vector.tensor_reduce(
                        out=s1, in_=sumabs_t[:, 0:nblk], axis=mybir.AxisListType.X,
                        op=mybir.AluOpType.add,
                    )
                else:
                    nc.vector.tensor_copy(out=s1, in_=sumabs_t[:, 0:1])
                ab = persist.tile(
                    [128, 1], FP32, tag=f"alphabar{m}", name=f"alphabar{m}"
                )
                nc.vector.tensor_tensor(
                    out=ab, in0=s1, in1=inv, op=mybir.AluOpType.mult
                )
                nc.vector.tensor_tensor(
                    out=ab, in0=ab, in1=gain_c[m], op=mybir.AluOpType.mult
                )
                nc.vector.tensor_scalar_mul(
                    out=ab, in0=ab, scalar1=ALPHA * WS_SCALE / KFAN
                )
                alphabar[m] = ab

            def transpose_cast(m, t, q, cast_engine):
                # 2 cin blocks (pair q) of tap t -> lhsT[:, t, q, :, m cols]
                ws3 = wsigns[m].rearrange("p (i t) -> p i t", t=TAPS)
                ps = psum_tr.tile([128, 2 * 128], BF16, tag="ptr", name="ptr")
                for h in range(2):
                    b = 2 * q + h
                    nc.tensor.transpose(
                        ps[:, h * 128 : (h + 1) * 128],
                        ws3[:, b * 128 : (b + 1) * 128, t],
                        ident,
                    )
                dst = lhsT[:, t, q, :, m * 128 : (m + 1) * 128]
                if cast_engine == "vector":
                    nc.vector.tensor_copy(out=dst, in_=ps)
                else:
                    nc.scalar.activation(
                        out=dst, in_=ps,
                        func=mybir.ActivationFunctionType.Copy,
                    )

            # ---- xsign: ACT path and GpSimd clamp path -------------------
            def xsign_act(n, c):
                dst = act_img[c // 2][n][:, c % 2, : HP * WP].rearrange(
                    "p (h w) -> p h w", w=WP
                )[:, 1 : 1 + H, 1 : 1 + W]
                nc.scalar.activation(
                    out=dst,
                    in_=xs_all[:, c, n],
                    func=mybir.ActivationFunctionType.Sign,
                    bias=b0_c[c],
                    scale=BETA,
                )

            def xsign_gps(n, c):
                dst = act_img[c // 2][n][:, c % 2, : HP * WP].rearrange(
                    "p (h w) -> p h w", w=WP
                )[:, 1 : 1 + H, 1 : 1 + W]
                tmp = epi.tile([128, H, W], FP32, tag="xtmp", name="xtmp")
                nc.gpsimd.tensor_scalar(
                    out=tmp, in0=xs_all[:, c, n], scalar1=b0_c[c], scalar2=BIG,
                    op0=mybir.AluOpType.add, op1=mybir.AluOpType.mult,
                )
                nc.gpsimd.tensor_scalar(
                    out=dst, in0=tmp, scalar1=1.0, scalar2=-1.0,
                    op0=mybir.AluOpType.min, op1=mybir.AluOpType.max,
                )

            # ---- HAM warmup: junk matmuls so the PE clock-gate opens to
            # 2.4 GHz before the real stream starts (transpose-mode does
            # not warm HAM, real MMs do; ~36 x 107ns fills the window
            # while the w chunk-0 DMA is in flight) ------------------------
            jp = psum_mm.tile(
                [128, ROWS_PER_TILE * W], FP32, tag="acc", name="junk"
            )
            for _ in range(70):
                nc.tensor.matmul(jp[:, :128], ident, ident, start=True, stop=True)

            # ---- prologue chain for chunk 0 ------------------------------
            # ACT order: img0 signs (x lands first), then weight signs,
            # then chunk0 q1 casts, then later-image signs, then abs.
            xsign_act(0, 0)
            xsign_act(0, 1)
            weight_stats(0)
            wsign_block(0, 0)
            wsign_block(0, 1)
            stdeps_sqrt(0)
            wsign_block(0, 2)
            wsign_block(0, 3)

            # transposes + casts: q0 via DVE, q1 via ACT
            for t in range(TAPS):
                transpose_cast(0, t, 0, "vector")
            for t in range(TAPS):
                transpose_cast(0, t, 1, "scalar")

            # gpsimd covers the c2/c3 act chunks for every image
            xsign_gps(0, 2)
            xsign_gps(0, 3)

            # x imgs 2-3 (gpsimd triggers, after the n0 clamp ops)
            x_dma(2, 0)
            x_dma(2, 1)
            x_dma(3, 0)
            x_dma(3, 1)

            # remaining act borders + xsign for n=1..3 (c01 ACT, c23 gps)
            for q in range(NPAIR):
                act_memset(q, 1)
            xsign_act(1, 0)
            xsign_act(1, 1)
            xsign_gps(1, 2)
            xsign_gps(1, 3)
            for q in range(NPAIR):
                act_memset(q, 2)
                act_memset(q, 3)
            xsign_act(2, 0)
            xsign_act(2, 1)
            xsign_gps(2, 2)
            xsign_gps(2, 3)

            # chunk0 |w-mean| on ACT after the n<=2 signs; alphabar gates
            # only the epilogue z ops, not the MM stream
            sumabs0 = stats.tile([128, NCH], FP32, tag="sumabs", name="sumabs0")
            wabs(0, [0, 1, 2, 3], sumabs0)
            alphabar_tail(0, sumabs0, NCH)

            xsign_act(3, 0)
            xsign_act(3, 1)
            xsign_gps(3, 2)
            xsign_gps(3, 3)

            # ---- conv ----------------------------------------------------
            def prep_a(m):
                # ACT part; weight_stats(m) is emitted earlier (DVE FIFO)
                for b in range(NCH):
                    wsign_block(m, b)
                stdeps_sqrt(m)
                sa = stats.tile([128, NCH], FP32, tag="sumabs", name=f"sa{m}")
                wabs(m, [0, 1, 2, 3], sa)
                alphabar_tail(m, sa, NCH)

            def prep_b(m):
                for t in range(TAPS):
                    transpose_cast(m, t, 0, "vector")
                    transpose_cast(m, t, 1, "vector")

            def conv_group(m, n):
                accs = []
                for h2 in range(NSPAT):
                    accs.append(
                        psum_mm.tile(
                            [128, ROWS_PER_TILE * W], FP32, tag="acc", name="acc"
                        )
                    )
                i = 0
                for q in range(NPAIR):
                    av = act_img[q][n][:, :, : HP * WP].rearrange(
                        "p h (r c) -> p h r c", c=WP
                    )
                    for t in range(TAPS):
                        dy, dx = t // 3, t % 3
                        for h2 in range(NSPAT):
                            y0 = h2 * ROWS_PER_TILE
                            rhs = av[
                                :, :, y0 + dy : y0 + dy + ROWS_PER_TILE,
                                dx : dx + W,
                            ]
                            nc.tensor.matmul(
                                accs[h2],
                                lhsT[:, t, q, :, m * 128 : (m + 1) * 128],
                                rhs,
                                start=(i == 0),
                                stop=(i == NPAIR * TAPS - 1),
                                perf_mode=mybir.MatmulPerfMode.DoubleRow,
                            )
                        i += 1
                for h2 in range(NSPAT):
                    y0 = h2 * ROWS_PER_TILE
                    accv = accs[h2].rearrange("p (h w) -> p h w", w=W)
                    res = xs_all[:, m, n, y0 : y0 + ROWS_PER_TILE, :]
                    # z = conv*alphabar + residual  (DVE, drains the bank)
                    z = epi.tile([128, ROWS_PER_TILE, W], FP32, tag="z", name="z")
                    nc.vector.scalar_tensor_tensor(
                        out=z, in0=accv, scalar=alphabar[m], in1=res,
                        op0=mybir.AluOpType.mult, op1=mybir.AluOpType.add,
                    )
                    # e = prelu(z + b1)  (ACT, per-channel alpha)
                    e = epi.tile([128, ROWS_PER_TILE, W], FP32, tag="e", name="e")
                    nc.scalar.activation(
                        out=e, in_=z,
                        func=mybir.ActivationFunctionType.Prelu,
                        bias=b1_c[m], alpha=pa_c[m],
                    )
                    # o = e + b2  (GpSimd; add+mult form — the op1=bypass
                    # ucode path measures ~7us/op vs ~0.6us for add+mult)
                    o = epi.tile([128, ROWS_PER_TILE, W], FP32, tag="oo", name="oo")
                    nc.gpsimd.tensor_scalar(
                        out=o, in0=e, scalar1=b2_c[m], scalar2=1.0,
                        op0=mybir.AluOpType.add, op1=mybir.AluOpType.mult,
                    )
                    nsplit = 2 if (m == NCH - 1 and n == N_PER - 1
                                   and h2 == NSPAT - 1) else 1
                    rr = ROWS_PER_TILE // nsplit
                    for s in range(nsplit):
                        nc.sync.dma_start(
                            out=out_ap[
                                n, m * 128 : (m + 1) * 128,
                                y0 + s * rr : y0 + (s + 1) * rr, :,
                            ],
                            in_=o[:, s * rr : (s + 1) * rr],
                        )

            for m in range(NCH):
                if m + 1 < NCH:
                    weight_stats(m + 1)
                for n in range(N_PER):
                    conv_group(m, n)
                    if m + 1 < NCH:
                        if n == 0:
                            prep_a(m + 1)
                        elif n == 1:
                            prep_b(m + 1)

    nc.finalize()
    return nc


_NC_CACHE = None


def _get_program():
    global _NC_CACHE
    if _NC_CACHE is None:
        _NC_CACHE = build_program()
    return _NC_CACHE


def kernel(**inputs):
    from concourse.bass_utils import run_bass_kernel_spmd

    x = np.ascontiguousarray(np.asarray(inputs["x"], dtype=np.float32))
    shared = {
        name: np.ascontiguousarray(np.asarray(inputs[name], dtype=np.float32))
        for name in (
            "conv_weight", "gain", "move0_bias", "move1_bias", "prelu_a",
            "move2_bias",
        )
    }
    nc = _get_program()
    in_maps = [
        {"x": x[i * N_PER : (i + 1) * N_PER], **shared} for i in range(N_CORES)
    ]
    res = run_bass_kernel_spmd(nc, in_maps, core_ids=list(range(N_CORES)))
    return np.concatenate([r["out"] for r in res.results], axis=0)


# revision 15
# speedup vs baseline: 1.8106x; 1.0462x over previous
"""Trainium2 Bass kernel for nn_BasicBlock (binary activation + binarized
weight-standardized 3x3 conv + residual + PReLU).

Contract: kernel(**inputs) takes FULL unsharded numpy inputs (keys as in
setup_inputs) and returns the FULL [32, 512, 28, 28] float32 output.
Internally shards the batch dim across 8 NeuronCores (4 images each); the
small conv weight + per-channel vectors are replicated.

Key math facts exploited:
- forward activations are sign(x*beta+b0) in {-1,0,1} and forward weights
  are sf[o]*gain[o]*sign(w_std) with sign in {-1,0,1}, so the conv
  contraction is exact in fp8 (products are +-1, fp32 PSUM accumulation);
  the per-channel scalar alpha*sf*gain folds into the epilogue.
- fp8e4 DoubleRow packs two contraction rows per PE cell (2 cin chunks per
  matmul), halving the matmul count.
- epilogue uses PReLU(v) = max(v, a*v) (valid for 0<=a<=1):
  out = max(z + b1 + b2, a*z + a*b1 + b2) with z = conv*alphabar + x,
  computed as z on DVE, u/o on GpSimd.

Schedule (from trace analysis of the 157us baseline):
- w chunk0 DMA pieces + x img0 land first (one logical queue each; a
  single dma_start is split across all 16 SDMA engines, so few big pieces
  beat many small ones; trigger cost ~0.6us each on the issuing engine).
- per-piece bn_stats pipeline with the DMA; wsign blocks 0,1 unlock the
  first transposes; casts (q0 on DVE, q1 on ACT) feed lhsT per (tap,pair)
  so the first conv matmul fires ~8us into the window instead of ~27us.
- conv loop is n-outer, h2-inner: each (q,t) weight is reused for 2
  matmuls so LDWEIGHTS always hides under the matmul stream.
- xsign: ACT for most; (n0,c2),(n0,c3),(n1,c2),(n1,c3) on GpSimd via a
  (x+b0)*1e30 clamp so ACT's wsign/abs stream isn't on the act critical
  path.
- weight prep for chunk m+1 is hooked into chunk m's conv at n==1/n==2.
"""

import numpy as np

import concourse.bass as bass
import concourse.mybir as mybir
import concourse.tile as tile
from concourse import bacc
from concourse.masks import make_identity

# problem constants (hardcoded per harness contract)
N_CORES = 8
N_PER = 4          # images per core (32 / 8)
C = 512            # Cin == Cout
H = W = 28
HP = WP = 30       # zero-padded spatial
TAPS = 9
KFAN = C * TAPS    # 4608 = fan-in per output channel
ALPHA = 0.2
BETA = 1.0
EPS = 1e-5
WS_SCALE = 1.0 / float(np.sqrt(KFAN))  # fan_in**-0.5
NCH = C // 128     # 4 channel chunks of 128
NPAIR = NCH // 2   # 2 DoubleRow pairs of chunks
ROWS_PER_TILE = 14 # output rows per matmul tile
NSPAT = H // ROWS_PER_TILE  # 2 spatial tiles per image
ACT_IMG = 912  # padded 30x30 image (900) + 12 slack: %16==0 for DoubleRow,
               # and covers the last tile's 420-run overhang
BIG = 1e30     # sign-via-clamp scale for the GpSimd xsign path

FP32 = mybir.dt.float32
BF16 = mybir.dt.bfloat16
FP8 = mybir.dt.float8e4

# weight DMA piece layout: 1024-column pieces (stats slices of 512 wait on
# piece j//2) -> 5 triggers per chunk instead of 9
W_PIECES = [(0, 1024), (1024, 1024), (2048, 1024), (3072, 1024), (4096, 512)]


def build_program():
    nc = bacc.Bacc(
        "TRN2",
        target_bir_lowering=False,
        debug=False,
        num_devices=1,
        num_swdge_queues=4,
    )
    x_h = nc.declare_dram_parameter("x", [N_PER, C, H, W], FP32, isOutput=False)
    w_h = nc.declare_dram_parameter("conv_weight", [C, C, 3, 3], FP32, isOutput=False)
    gain_h = nc.declare_dram_parameter("gain", [C], FP32, isOutput=False)
    b0_h = nc.declare_dram_parameter("move0_bias", [C], FP32, isOutput=False)
    b1_h = nc.declare_dram_parameter("move1_bias", [C], FP32, isOutput=False)
    pa_h = nc.declare_dram_parameter("prelu_a", [C], FP32, isOutput=False)
    b2_h = nc.declare_dram_parameter("move2_bias", [C], FP32, isOutput=False)
    out_h = nc.declare_dram_parameter("out", [N_PER, C, H, W], FP32, isOutput=True)

    x_ap = x_h[:, :, :, :]
    w_ap = w_h[:, :, :, :]
    out_ap = out_h[:, :, :, :]

    with tile.TileContext(nc) as tc:
        with (
            tc.tile_pool(name="persist", bufs=1) as persist,
            tc.tile_pool(name="scratch", bufs=2) as scratch,
            tc.tile_pool(name="stats", bufs=4) as stats,
            tc.tile_pool(name="epi", bufs=4) as epi,
            tc.tile_pool(name="psum_mm", bufs=6, space="PSUM") as psum_mm,
            tc.tile_pool(name="psum_tr", bufs=2, space="PSUM") as psum_tr,
        ):
            # ---- w chunk 0 DMA first: the whole kernel gates on it ------
            w_flat = w_ap.rearrange("o i a b -> o (i a b)")
            w_tiles = []

            def w_dma(m):
                wt = scratch.tile([128, KFAN], FP32, tag="wtile", name=f"wt{m}")
                for (c0, cw) in W_PIECES:
                    nc.sync.dma_start(
                        out=wt[:, c0 : c0 + cw],
                        in_=w_flat[m * 128 : (m + 1) * 128, c0 : c0 + cw],
                    )
                w_tiles.append(wt)

            # tiny per-channel vectors ride the scalar HWDGE ring (separate
            # physical ring from sync, so they don't delay w chunk0; ~2us
            # fixed cost each serialized would cost w0 10us on sync)
            def load_vec(dram_h, name):
                t = persist.tile([128, NCH], FP32, tag=name, name=name)
                nc.scalar.dma_start(
                    out=t, in_=dram_h[:].rearrange("(c p) -> p c", p=128)
                )
                return [t[:, c : c + 1] for c in range(NCH)]

            gain_c = load_vec(gain_h, "gain")
            b0_c = load_vec(b0_h, "b0")
            b1_c = load_vec(b1_h, "b1")
            pa_c = load_vec(pa_h, "pa")
            b2_c = load_vec(b2_h, "b2")

            w_dma(0)

            # ---- gpsimd: identity, act borders (img0), x img0 ------------
            ident = persist.tile([128, 128], BF16, tag="ident")
            make_identity(nc, ident)

            act_img = []
            for q in range(NPAIR):
                row = []
                for n in range(N_PER):
                    ap_t = persist.tile(
                        [128, 2, ACT_IMG], FP8, tag=f"act{q}_{n}", name=f"act{q}_{n}"
                    )
                    row.append(ap_t)
                act_img.append(row)

            def act_memset(q, n):
                ap_t = act_img[q][n]
                nc.gpsimd.memset(ap_t[:, :, 0:WP], 0.0)
                nc.gpsimd.memset(ap_t[:, :, 29 * WP : ACT_IMG], 0.0)
                mid = ap_t[:, :, WP : 29 * WP].rearrange("p h (r c) -> p h r c", c=WP)
                nc.gpsimd.memset(mid[:, :, :, 0:1], 0.0)
                nc.gpsimd.memset(mid[:, :, :, 29:30], 0.0)

            for q in range(NPAIR):
                act_memset(q, 0)

            xs_all = persist.tile([128, NCH, N_PER, H, W], FP32, tag="xs", name="xs")
            xr = x_ap.rearrange("n (cc p) h w -> p cc n (h w)", p=128)

            def x_dma(n, chalf):  # chalf: 0 -> chunks 0,1 ; 1 -> chunks 2,3
                c0 = 2 * chalf
                nc.gpsimd.dma_start(
                    out=xs_all[:, c0 : c0 + 2, n].rearrange("p c h w -> p c (h w)"),
                    in_=xr[:, c0 : c0 + 2, n, :],
                )

            x_dma(0, 0)
            x_dma(0, 1)

            # ---- sync: w chunks 1-3 --------------------------------------
            for m in range(1, NCH):
                w_dma(m)

            # gpsimd: x images 1-3 (split 2/3 for finer completion)
            x_dma(1, 0)
            x_dma(1, 1)

            # ---- weight stats chunk 0 (DVE; slices pipeline with DMA) ----
            lhsT = persist.tile(
                [128, TAPS, NPAIR, 2, C], FP8, tag="lhsT", name="lhsT"
            )
            alphabar = {}
            wsigns = {}
            mvs = {}

            def weight_stats(m):
                wt = w_tiles[m]
                st = stats.tile([128, TAPS, 6], FP32, tag="bnst", name="bnst")
                wt3 = wt.rearrange("p (a b) -> p a b", b=512)
                for sg in range(TAPS):
                    nc.vector.bn_stats(out=st[:, sg, :], in_=wt3[:, sg, :])
                mv = stats.tile([128, 2], FP32, tag="bnagg", name="bnagg")
                nc.vector.bn_aggr(out=mv, in_=st)
                negmean = stats.tile([128, 1], FP32, tag="negmean", name="negmean")
                nc.vector.tensor_scalar_mul(out=negmean, in0=mv[:, 0:1], scalar1=-1.0)
                mvs[m] = (mv, negmean)

            def wsign_block(m, b):
                # sign(w - mean) -> bf16 for cin block b
                if b == 0:
                    ws = scratch.tile([128, KFAN], BF16, tag="wsign", name=f"ws{m}")
                    wsigns[m] = ws
                ws = wsigns[m]
                nc.scalar.activation(
                    out=ws[:, b * 1152 : (b + 1) * 1152],
                    in_=w_tiles[m][:, b * 1152 : (b + 1) * 1152],
                    func=mybir.ActivationFunctionType.Sign,
                    bias=mvs[m][1],
                )

            def stdeps_sqrt(m):
                sd = stats.tile([128, 1], FP32, tag="stdeps", name=f"sd{m}")
                nc.scalar.activation(
                    out=sd, in_=mvs[m][0][:, 1:2],
                    func=mybir.ActivationFunctionType.Sqrt,
                )
                mvs[m] = (mvs[m][0], mvs[m][1], sd)

            def wabs(m, blocks, sumabs_t):
                # |w - mean| accumulated per block (overwrites w tile)
                wt = w_tiles[m]
                for i, b in enumerate(blocks):
                    nc.scalar.activation(
                        out=wt[:, b * 1152 : (b + 1) * 1152],
                        in_=wt[:, b * 1152 : (b + 1) * 1152],
                        func=mybir.ActivationFunctionType.Abs,
                        bias=mvs[m][1],
                        accum_out=sumabs_t[:, b : b + 1],
                    )

            def alphabar_tail(m, sumabs_t, nblk):
                sd = mvs[m][2]
                sde = stats.tile([128, 1], FP32, tag="sde", name=f"sde{m}")
                nc.vector.tensor_scalar_add(out=sde, in0=sd, scalar1=EPS)
                inv = stats.tile([128, 1], FP32, tag="inv", name=f"inv{m}")
                nc.vector.reciprocal(out=inv, in_=sde)
                s1 = stats.tile([128, 1], FP32, tag="s1", name=f"s1{m}")
                if nblk > 1:
                    nc.vector.tensor_reduce(
                        out=s1, in_=sumabs_t[:, 0:nblk], axis=mybir.AxisListType.X,
                        op=mybir.AluOpType.add,
                    )
                else:
                    nc.vector.tensor_copy(out=s1, in_=sumabs_t[:, 0:1])
                ab = persist.tile(
                    [128, 1], FP32, tag=f"alphabar{m}", name=f"alphabar{m}"
                )
                nc.vector.tensor_tensor(
                    out=ab, in0=s1, in1=inv, op=mybir.AluOpType.mult
                )
                nc.vector.tensor_tensor(
                    out=ab, in0=ab, in1=gain_c[m], op=mybir.AluOpType.mult
                )
                nc.vector.tensor_scalar_mul(
                    out=ab, in0=ab, scalar1=ALPHA * WS_SCALE / KFAN
                )
                alphabar[m] = ab

            def transpose_cast(m, t, q, cast_engine):
                # 2 cin blocks (pair q) of tap t -> lhsT[:, t, q, :, m cols]
                ws3 = wsigns[m].rearrange("p (i t) -> p i t", t=TAPS)
                ps = psum_tr.tile([128, 2 * 128], BF16, tag="ptr", name="ptr")
                for h in range(2):
                    b = 2 * q + h
                    nc.tensor.transpose(
                        ps[:, h * 128 : (h + 1) * 128],
                        ws3[:, b * 128 : (b + 1) * 128, t],
                        ident,
                    )
                dst = lhsT[:, t, q, :, m * 128 : (m + 1) * 128]
                if cast_engine == "vector":
                    nc.vector.tensor_copy(out=dst, in_=ps)
                else:
                    nc.scalar.activation(
                        out=dst, in_=ps,
                        func=mybir.ActivationFunctionType.Copy,
                    )

            # ---- xsign: ACT path and GpSimd clamp path -------------------
            def xsign_act(n, c):
                dst = act_img[c // 2][n][:, c % 2, : HP * WP].rearrange(
                    "p (h w) -> p h w", w=WP
                )[:, 1 : 1 + H, 1 : 1 + W]
                nc.scalar.activation(
                    out=dst,
                    in_=xs_all[:, c, n],
                    func=mybir.ActivationFunctionType.Sign,
                    bias=b0_c[c],
                    scale=BETA,
                )

            def xsign_gps(n, c):
                dst = act_img[c // 2][n][:, c % 2, : HP * WP].rearrange(
                    "p (h w) -> p h w", w=WP
                )[:, 1 : 1 + H, 1 : 1 + W]
                tmp = epi.tile([128, H, W], FP32, tag="xtmp", name="xtmp")
                nc.gpsimd.tensor_scalar(
                    out=tmp, in0=xs_all[:, c, n], scalar1=b0_c[c], scalar2=BIG,
                    op0=mybir.AluOpType.add, op1=mybir.AluOpType.mult,
                )
                nc.gpsimd.tensor_scalar(
                    out=dst, in0=tmp, scalar1=1.0, scalar2=-1.0,
                    op0=mybir.AluOpType.min, op1=mybir.AluOpType.max,
                )

            # ---- HAM warmup: junk matmuls so the PE clock-gate opens to
            # 2.4 GHz before the real stream starts (transpose-mode does
            # not warm HAM, real MMs do; ~36 x 107ns fills the window
            # while the w chunk-0 DMA is in flight) ------------------------
            jp = psum_mm.tile(
                [128, ROWS_PER_TILE * W], FP32, tag="acc", name="junk"
            )
            for _ in range(70):
                nc.tensor.matmul(jp[:, :128], ident, ident, start=True, stop=True)

            # ---- prologue chain for chunk 0 ------------------------------
            # ACT order: img0 signs (x lands first), then weight signs,
            # then chunk0 q1 casts, then later-image signs, then abs.
            xsign_act(0, 0)
            xsign_act(0, 1)
            weight_stats(0)
            wsign_block(0, 0)
            wsign_block(0, 1)
            stdeps_sqrt(0)
            wsign_block(0, 2)
            wsign_block(0, 3)

            # transposes + casts: q0 via DVE, q1 via ACT
            for t in range(TAPS):
                transpose_cast(0, t, 0, "vector")
            for t in range(TAPS):
                transpose_cast(0, t, 1, "scalar")

            # gpsimd covers the c2/c3 act chunks for every image
            xsign_gps(0, 2)
            xsign_gps(0, 3)

            # x imgs 2-3 (gpsimd triggers, after the n0 clamp ops)
            x_dma(2, 0)
            x_dma(2, 1)
            x_dma(3, 0)
            x_dma(3, 1)

            # remaining act borders + xsign for n=1..3 (c01 ACT, c23 gps)
            for q in range(NPAIR):
                act_memset(q, 1)
            xsign_act(1, 0)
            xsign_act(1, 1)
            xsign_gps(1, 2)
            xsign_gps(1, 3)
            for q in range(NPAIR):
                act_memset(q, 2)
                act_memset(q, 3)
            xsign_act(2, 0)
            xsign_act(2, 1)
            xsign_gps(2, 2)
            xsign_gps(2, 3)

            # chunk0 |w-mean| on ACT after the n<=2 signs; alphabar gates
            # only the epilogue z ops, not the MM stream
            sumabs0 = stats.tile([128, NCH], FP32, tag="sumabs", name="sumabs0")
            wabs(0, [0, 1, 2, 3], sumabs0)
            alphabar_tail(0, sumabs0, NCH)

            xsign_act(3, 0)
            xsign_act(3, 1)
            xsign_gps(3, 2)
            xsign_gps(3, 3)

            # ---- conv ----------------------------------------------------
            def prep_a(m):
                # ACT part; weight_stats(m) is emitted earlier (DVE FIFO)
                for b in range(NCH):
                    wsign_block(m, b)
                stdeps_sqrt(m)
                sa = stats.tile([128, NCH], FP32, tag="sumabs", name=f"sa{m}")
                wabs(m, [0, 1, 2, 3], sa)
                alphabar_tail(m, sa, NCH)

            def prep_b(m):
                for t in range(TAPS):
                    transpose_cast(m, t, 0, "vector")
                    transpose_cast(m, t, 1, "vector")

            def conv_group(m, n):
                accs = []
                for h2 in range(NSPAT):
                    accs.append(
                        psum_mm.tile(
                            [128, ROWS_PER_TILE * W], FP32, tag="acc", name="acc"
                        )
                    )
                i = 0
                for q in range(NPAIR):
                    av = act_img[q][n][:, :, : HP * WP].rearrange(
                        "p h (r c) -> p h r c", c=WP
                    )
                    for t in range(TAPS):
                        dy, dx = t // 3, t % 3
                        for h2 in range(NSPAT):
                            y0 = h2 * ROWS_PER_TILE
                            rhs = av[
                                :, :, y0 + dy : y0 + dy + ROWS_PER_TILE,
                                dx : dx + W,
                            ]
                            nc.tensor.matmul(
                                accs[h2],
                                lhsT[:, t, q, :, m * 128 : (m + 1) * 128],
                                rhs,
                                start=(i == 0),
                                stop=(i == NPAIR * TAPS - 1),
                                perf_mode=mybir.MatmulPerfMode.DoubleRow,
                            )
                        i += 1
                for h2 in range(NSPAT):
                    y0 = h2 * ROWS_PER_TILE
                    accv = accs[h2].rearrange("p (h w) -> p h w", w=W)
                    res = xs_all[:, m, n, y0 : y0 + ROWS_PER_TILE, :]
                    # z = conv*alphabar + residual  (DVE, drains the bank)
                    z = epi.tile([128, ROWS_PER_TILE, W], FP32, tag="z", name="z")
                    nc.vector.scalar_tensor_tensor(
                        out=z, in0=accv, scalar=alphabar[m], in1=res,
                        op0=mybir.AluOpType.mult, op1=mybir.AluOpType.add,
                    )
                    # e = prelu(z + b1)  (ACT, per-channel alpha)
                    e = epi.tile([128, ROWS_PER_TILE, W], FP32, tag="e", name="e")
                    nc.scalar.activation(
                        out=e, in_=z,
                        func=mybir.ActivationFunctionType.Prelu,
                        bias=b1_c[m], alpha=pa_c[m],
                    )
                    # o = e + b2  (GpSimd; add+mult form — the op1=bypass
                    # ucode path measures ~7us/op vs ~0.6us for add+mult)
                    o = epi.tile([128, ROWS_PER_TILE, W], FP32, tag="oo", name="oo")
                    nc.gpsimd.tensor_scalar(
                        out=o, in0=e, scalar1=b2_c[m], scalar2=1.0,
                        op0=mybir.AluOpType.add, op1=mybir.AluOpType.mult,
                    )
                    nsplit = 2 if (m == NCH - 1 and n == N_PER - 1
                                   and h2 == NSPAT - 1) else 1
                    rr = ROWS_PER_TILE // nsplit
                    for s in range(nsplit):
                        nc.sync.dma_start(
                            out=out_ap[
                                n, m * 128 : (m + 1) * 128,
                                y0 + s * rr : y0 + (s + 1) * rr, :,
                            ],
                            in_=o[:, s * rr : (s + 1) * rr],
                        )

            for m in range(NCH):
                if m + 1 < NCH:
                    weight_stats(m + 1)
                for n in range(N_PER):
                    conv_group(m, n)
                    if m + 1 < NCH:
                        if n == 0:
                            prep_a(m + 1)
                        elif n == 1:
                            prep_b(m + 1)

    nc.finalize()
    return nc


_NC_CACHE = None


def _get_program():
    global _NC_CACHE
    if _NC_CACHE is None:
        _NC_CACHE = build_program()
    return _NC_CACHE


def kernel(**inputs):
    from concourse.bass_utils import run_bass_kernel_spmd

    x = np.ascontiguousarray(np.asarray(inputs["x"], dtype=np.float32))
    shared = {
        name: np.ascontiguousarray(np.asarray(inputs[name], dtype=np.float32))
        for name in (
            "conv_weight", "gain", "move0_bias", "move1_bias", "prelu_a",
            "move2_bias",
        )
    }
    nc = _get_program()
    in_maps = [
        {"x": x[i * N_PER : (i + 1) * N_PER], **shared} for i in range(N_CORES)
    ]
    res = run_bass_kernel_spmd(nc, in_maps, core_ids=list(range(N_CORES)))
    return np.concatenate([r["out"] for r in res.results], axis=0)
